# revision 44
# baseline (speedup 1.0000x reference)
"""Trainium2 Bass kernel for nn_Diffuser_78331613544465.

Math (per graph b of B=8, N=1024):
    A   = adj (mask is all-ones in the graded setup; general mask handled host-side)
    P   = A / max(rowsum(A), 1)
    out[i,j,:] = relu([I, P, P2, P4][i,j,:] @ w1 + b1) @ w2 + b2   (P2=P@P, P4=P2@P2)

Device strategy: data-parallel over B — one graph per NeuronCore (8 cores).
On-chip work happens in the TRANSPOSED domain (Q = P^T).  Because A is
symmetric, both P = D^-1 A (row scale) and Q = A D^-1 (col scale) come from
cheap elementwise scalings of A, and the power chain needs NO PE transposes:
    Q2 = P^T Q,  P2 = Q^T P,  Q4 = P2^T Q2      (matmul(lhsT=X, rhs=Y) = X^T Y)

Edge MLP layer 1 runs as 4 CONCURRENT row-tiled matmuls (tile_position=(32r,0),
K=32 = 4 stacks x 8 j's, M=128 = 8 j x 16 hid) per 512-i half — 8 matmuls per
32-j group instead of 32.  PSUM h tiles are [128,1024] (two banks, two j-blocks)
so the relu+bias evacuation runs at FD=1024, alternating Vector/Scalar (the only
two engines with PSUM access).  Layer 2 is K=128 block-diagonal with the two
j-blocks of a pair issued as concurrent column tiles (tile_position=(0,0)/(0,64)
against a duplicated weight [w2blk|w2blk]).

Channels are staged through a DRAM interleave il4[jb, s, jj, i] (jb = j-block,
s = 0:I (host-uploaded), 1:Q, 2:Q2, 3:Q4) laid out so that every group stage
load is one CONTIGUOUS 64KB DMA per j-block and every band spill is a simple
2-level affine pattern — no expensive gather descriptors.

The MLP groups are EMITTED INTERLEAVED into the second power-chain square with
a one-band lag, so the Vector/Scalar evacuation work (the MLP bottleneck)
overlaps the PE-bound power chain.

The [16j x 8o, i] PSUM result is evacuated as fp16 and the HOST un-transposes
— this keeps every output-DMA descriptor a full 2KB partition line.

kernel(**inputs) takes FULL inputs, shards over 8 cores, returns FULL output.
"""

import os
import numpy as np

B, N, P = 8, 1024, 128
HID, HEADS, NSTACK = 16, 8, 4
NT = N // P          # 8 row-tiles
JBLK = 8             # j rows per MLP block
NJB = N // JBLK      # 128 j-blocks
IC = 512             # i-chunk (matmul free dim)
NIC = N // IC        # 2
NGRP = N // 32       # 32 j-groups of 32 j's (4 j-blocks)

_CACHE = {}
LAST_RESULTS = None


def _emit(nc, tc, ctx, mm_dt):
    from concourse import mybir

    f32 = mybir.dt.float32
    add = mybir.AluOpType.add
    amax = mybir.AluOpType.max
    mult = mybir.AluOpType.mult
    relu_fn = mybir.ActivationFunctionType.Relu

    adj = nc.declare_dram_parameter("adj", [N, N], mm_dt, isOutput=False)
    w1sel_d = nc.declare_dram_parameter("w1sel", [P, P], mm_dt, isOutput=False)
    w2blk_d = nc.declare_dram_parameter("w2blk", [P, JBLK * HEADS], mm_dt, isOutput=False)
    b1rep_d = nc.declare_dram_parameter("b1rep", [P, 1], f32, isOutput=False)
    idn32_d = nc.declare_dram_parameter("idn32", [P, P], f32, isOutput=False)
    idnil_d = nc.declare_dram_parameter("idnil", [NJB, JBLK, N], mm_dt, isOutput=False)
    # device-natural output: [jb-pair, (16j x 8o) partition, i] in fp16;
    # host transposes to [i, j, o] and casts to f32
    out = nc.declare_dram_parameter("out", [NJB // 2, P, N], mm_dt, isOutput=True)

    small = ctx.enter_context(tc.tile_pool(name="small", bufs=1))
    big = ctx.enter_context(tc.tile_pool(name="big", bufs=1))
    spool = ctx.enter_context(tc.tile_pool(name="spool", bufs=8))
    rpool = ctx.enter_context(tc.tile_pool(name="rpool", bufs=20))
    ppool = ctx.enter_context(tc.tile_pool(name="ppool", bufs=8))
    dram = ctx.enter_context(tc.tile_pool(name="dram", bufs=1, space="DRAM"))
    # PSUM budget (8 banks): h 6x[128,512] single-bank slots (shared with the
    # power-chain accumulators; one slot per j-block so slots recycle
    # one-by-one instead of in 2-bank pairs), o 2x[128,512]=2
    h_ps = ctx.enter_context(tc.tile_pool(name="h_ps", bufs=6, space="PSUM"))
    o_ps = ctx.enter_context(tc.tile_pool(name="o_ps", bufs=2, space="PSUM"))

    # persistent matrices, one [128, 1024] tile per 128-row band
    Af = [big.tile([P, N], mm_dt, name=f"Af{t}", tag=f"Af{t}") for t in range(NT)]
    Pf = [big.tile([P, N], mm_dt, name=f"Pf{t}", tag=f"Pf{t}") for t in range(NT)]
    Qf = [big.tile([P, N], mm_dt, name=f"Qf{t}", tag=f"Qf{t}") for t in range(NT)]
    Q2f = [big.tile([P, N], mm_dt, name=f"Q2f{t}", tag=f"Q2f{t}") for t in range(NT)]
    P2f = [big.tile([P, N], mm_dt, name=f"P2f{t}", tag=f"P2f{t}") for t in range(NT)]
    Q4f = [big.tile([P, N], mm_dt, name=f"Q4f{t}", tag=f"Q4f{t}") for t in range(NT)]
    invrep = big.tile([P, N], f32, tag="invrep")
    # DRAM channel-interleave [jb, s, jj, i]: s=0 identity (host), 1..3=Q,Q2,Q4
    il4 = dram.tile([NJB, NSTACK, JBLK, N], mm_dt, tag="il4")

    # ---- constants / weights (host-prepared; one DMA each) -----------------
    idn32 = small.tile([P, P], f32, tag="idn32")
    nc.gpsimd.dma_start(idn32[:], idn32_d[:])
    ones1 = small.tile([1, P], f32, tag="ones1")
    nc.vector.memset(ones1[:], 1.0)
    w1sel = small.tile([P, P], mm_dt, tag="w1sel")
    nc.gpsimd.dma_start(w1sel[:], w1sel_d[:])
    w2blk = small.tile([P, JBLK * HEADS], mm_dt, tag="w2blk")
    nc.gpsimd.dma_start(w2blk[:], w2blk_d[:])
    b1rep = small.tile([P, 1], f32, tag="b1rep")
    nc.gpsimd.dma_start(b1rep[:], b1rep_d[:])

    # ---- phase 1: load adj (fp16 via DMA cast, 4 queue-spread chunks per
    # band), deg -> invdeg, P ------------------------------------------------
    invcol = small.tile([P, NT], f32, tag="invcol")
    for t in range(NT):
        # adj is pre-cast to fp16 by the host: cheap HWDGE loads, 2 queues/band
        for q in range(2):
            eng = nc.scalar if (2 * t + q) % 2 == 0 else nc.sync
            eng.dma_start(
                Af[t][:, 512 * q:512 * (q + 1)],
                adj[P * t:P * (t + 1), 512 * q:512 * (q + 1)],
            )
        deg = small.tile([P, 1], f32, tag=f"deg{t}")
        nc.vector.tensor_reduce(
            deg[:], Af[t][:], axis=mybir.AxisListType.X, op=add,
        )
        degc = small.tile([P, 1], f32, tag=f"degc{t}")
        nc.vector.tensor_scalar_max(degc[:], deg[:], 1.0)
        nc.vector.reciprocal(invcol[:, t:t + 1], degc[:])
        # P = A * invdeg[row]  (per-partition scale on the scalar engine)
        nc.scalar.mul(Pf[t][:], Af[t][:], invcol[:, t:t + 1])

    # identity channel of the interleave: 16 chunked DRAM->DRAM copies,
    # emitted AFTER the adj loads so they don't delay band arrival (only
    # needed once the MLP stage loads begin)
    for ch in range(16):
        nc.sync.dma_start(
            il4[8 * ch:8 * (ch + 1), 0:1, :, :],
            idnil_d[8 * ch:8 * (ch + 1), :, :],
        )

    # invrep[p, c] = invdeg(row c) for all p  (transpose + broadcast via PE)
    invrow = small.tile([1, N], f32, tag="invrow")
    for t in range(NT):
        ptp = o_ps.tile([P, IC], f32, tag="O")
        nc.tensor.transpose(ptp[0:1, 0:P], invcol[:, t:t + 1], idn32[:])
        nc.scalar.copy(invrow[0:1, P * t:P * (t + 1)], ptp[0:1, 0:P])
    for half in range(2):
        pb = o_ps.tile([P, IC], f32, tag="O")
        for k in range(4):
            c = 4 * half + k
            nc.tensor.matmul(
                pb[:, P * k:P * (k + 1)], ones1[:], invrow[0:1, P * c:P * (c + 1)],
                start=True, stop=True,
            )
        nc.scalar.copy(invrep[:, IC * half:IC * (half + 1)], pb[:])

    def spill(si, t, src, split=False):
        # optional split: two half-band DMAs on separate queues halve the
        # DRAM-visibility latency (used for Q4, which gates the MLP stage)
        if split:
            nc.sync.dma_start(
                il4[16 * t:16 * t + 8, si:si + 1, :, :], src[0:64, :])
            nc.sync.dma_start(
                il4[16 * t + 8:16 * t + 16, si:si + 1, :, :], src[64:128, :])
        else:
            nc.sync.dma_start(il4[16 * t:16 * (t + 1), si:si + 1, :, :], src[:])

    # PE warmup: keep the HAM activity monitor hot so the first square runs
    # at 2.4 GHz from its first matmul (overlaps the tail of the adj loads)
    warm = o_ps.tile([P, IC], f32, tag="O")
    for i in range(20):
        nc.tensor.matmul(
            warm[:], Af[0][:, 0:P], Af[0][:, 0:IC],
            start=(i == 0), stop=(i == 19),
        )

    # ---- power chain (no transposes; M3 := A D^-1 A is symmetric, so ONE
    # square yields both Q2 = M3 D^-1 (col scale) and P2 = D^-1 M3 (row
    # scale); then Q4 = Q2^2 = P2^T Q2) -------------------------------------
    for al in range(NT):
        for be in range(NIC):
            mm = h_ps.tile([P, IC], f32, tag="H")
            for g in range(NT):
                nc.tensor.matmul(
                    mm[:],
                    Af[g][:, P * al:P * (al + 1)],
                    Pf[g][:, IC * be:IC * (be + 1)],
                    start=(g == 0), stop=(g == NT - 1),
                )
            nc.vector.tensor_tensor(
                Q2f[al][:, IC * be:IC * (be + 1)], mm[:],
                invrep[:, IC * be:IC * (be + 1)], op=mult,
            )
            nc.scalar.mul(
                P2f[al][:, IC * be:IC * (be + 1)], mm[:], invcol[:, al:al + 1],
            )
        spill(2, al, Q2f[al])

    # ---- edge MLP, software-pipelined over groups (32 j's each) ------------
    stage_tiles = {}
    rt_tiles = {}

    def emit_stage(G):
        stage = spool.tile([P, N], mm_dt, tag="S")
        # ONE contiguous 256KB DMA per group: partitions (r, s, jj)
        eng = nc.sync if G % 2 == 0 else nc.scalar
        eng.dma_start(stage[:], il4[4 * G:4 * (G + 1), :, :, :])
        stage_tiles[G] = stage

    def emit_l1(G):
        stage = stage_tiles.pop(G)
        rts = {}
        for ic in range(NIC):
            # 16 concurrent 32x32 PE tiles (the only tile size that packs):
            # tile (r,c) computes j-pair (2c,2c+1) x 16 hid of block 4G+r,
            # one single-bank PSUM slot per j-block r
            hs = [h_ps.tile([P, IC], f32, name=f"h{r}", tag="H")
                  for r in range(4)]
            for r in range(4):
                for c2 in range(2):
                    nc.tensor.matmul(
                        hs[r][64 * c2:64 * (c2 + 1), :],
                        w1sel[32 * r:32 * (r + 1), 64 * c2:64 * (c2 + 1)],
                        stage[32 * r:32 * (r + 1), IC * ic:IC * (ic + 1)],
                        start=True, stop=True, tile_position=(32 * r, 64 * c2),
                    )
            # relu+bias evacuation per block, alternating V/S so slots free
            # one-by-one on both engines
            for r in range(4):
                rt = rpool.tile([P, IC], mm_dt, name=f"rt{r}", tag="R")
                if r % 2 == 0:
                    nc.vector.tensor_scalar(rt[:], hs[r][:], b1rep[:], 0.0,
                                            add, amax)
                else:
                    nc.scalar.activation(rt[:], hs[r][:], relu_fn,
                                         bias=b1rep[:], scale=1.0)
                rts[(r, ic)] = rt
        rt_tiles[G] = rts

    def emit_l2(G):
        rts = rt_tiles.pop(G)
        for rp in range(2):
            psout = ppool.tile([P, N], mm_dt, tag="PS")
            pos = [o_ps.tile([P, IC], f32, name=f"po{i}", tag="O")
                   for i in range(NIC)]
            # emission order A(ic0), A(ic1), B(ic0), B(ic1): consecutive MMs
            # share lhsT AP and tile_position, giving LDW reuse a chance
            nc.tensor.matmul(
                pos[0][0:64, :], w2blk[:], rts[(2 * rp, 0)][:],
                start=True, stop=True, tile_position=(0, 0),
            )
            nc.tensor.matmul(
                pos[1][0:64, :], w2blk[:], rts[(2 * rp, 1)][:],
                start=True, stop=True, tile_position=(0, 0),
            )
            nc.tensor.matmul(
                pos[0][64:128, :], w2blk[:], rts[(2 * rp + 1, 0)][:],
                start=True, stop=True, tile_position=(0, 64),
            )
            nc.tensor.matmul(
                pos[1][64:128, :], w2blk[:], rts[(2 * rp + 1, 1)][:],
                start=True, stop=True, tile_position=(0, 64),
            )
            for ic in range(NIC):
                # split the 4 po evacs 2/2 across V and S
                if (rp + ic) % 2 == 0:
                    nc.vector.tensor_scalar_add(
                        psout[:, IC * ic:IC * (ic + 1)], pos[ic][:], 0.0,
                    )
                else:
                    nc.scalar.copy(psout[:, IC * ic:IC * (ic + 1)], pos[ic][:])
            # out DMA on gpsimd: idle during the MLP and its (possibly
            # waiting) trigger blocks nothing else
            nc.gpsimd.dma_start(out[2 * G + rp], psout[:])

    prog = [0, 0, 0]  # stage / l1 / l2 emission pointers

    def pump(stage_t, l1_t, l2_t):
        # round-robin the three pipeline stages up to their targets; cap the
        # stage prefetch at spool depth minus one past the consumer so stage
        # triggers never stall at a DMA queue head (head-of-line blocking)
        while (min(stage_t, prog[1] + 7) > prog[0]
               or prog[1] < l1_t or prog[2] < l2_t):
            if prog[0] < min(stage_t, prog[1] + 7):
                emit_stage(prog[0])
                prog[0] += 1
            if prog[1] < l1_t:
                emit_l1(prog[1])
                prog[1] += 1
            if prog[2] < l2_t:
                emit_l2(prog[2])
                prog[2] += 1

    # ---- second square, MLP pipeline interleaved as bands become ready -----
    for al in range(NT):
        for be in range(NIC):
            mm = h_ps.tile([P, IC], f32, tag="H")
            for g in range(NT):
                nc.tensor.matmul(
                    mm[:],
                    P2f[g][:, P * al:P * (al + 1)],
                    Q2f[g][:, IC * be:IC * (be + 1)],
                    start=(g == 0), stop=(g == NT - 1),
                )
            if be == 0:
                nc.vector.tensor_scalar_add(Q4f[al][:, 0:IC], mm[:], 0.0)
            else:
                nc.scalar.copy(Q4f[al][:, IC:N], mm[:])
        spill(3, al, Q4f[al], split=True)
        # Q channel for band al: V has slack during the second square, and
        # doing it here keeps the first square's scale consumers unblocked
        nc.vector.tensor_tensor(Qf[al][:], Af[al][:], invrep[:], op=mult)
        spill(1, al, Qf[al])
        # groups of band al are now ready: prefetch their stage DMAs (pure
        # background traffic), but keep ALL matmul/evac work out of the
        # power chain - interleaving couples the phases through PSUM slots
        # and breaks correctness (observed 7e-2 rel err when L1 waves share
        # the pool with in-flight accumulations)
        pump(min(4 * (al + 1), NGRP), 0, 0)
    pump(NGRP, NGRP, NGRP)


def _build(mm_dtype_name="float16"):
    key = mm_dtype_name
    if key in _CACHE:
        return _CACHE[key]
    from contextlib import ExitStack
    import concourse.tile as tile
    from concourse import bacc, mybir

    nc = bacc.Bacc()
    with tile.TileContext(nc) as tc:
        with ExitStack() as ctx:
            _emit(nc, tc, ctx, getattr(mybir.dt, mm_dtype_name))
    nc.compile()
    _CACHE[key] = nc
    return nc


def _install_ntff_shim():
    """The agent image's antenv lacks axon_hooks; provide it and register the
    ctypes NTFF hook so run_bass_kernel_spmd(trace=True) can profile."""
    import sys
    import types

    if "antenv.axon_hooks" in sys.modules:
        return
    mod = types.ModuleType("antenv.axon_hooks")
    mod._hook = None
    mod.set_axon_ntff_profile_hook = lambda h: setattr(mod, "_hook", h)
    mod.get_axon_ntff_profile_hook = lambda: mod._hook
    sys.modules["antenv.axon_hooks"] = mod
    try:
        from trn_agent_boot.trn_boot import _ntff_profile_via_ctypes

        mod._hook = _ntff_profile_via_ctypes("/opt/axon/libaxon_pjrt.so")
    except Exception as e:  # degrade to no-trace
        print(f"ntff shim install failed: {e}")


def _host_tensors(w1, b1, w2, np_mm):
    # selector weights: tile (r,c) maps staged rows (s, jj) -> (jj', hid) of
    # j-pair (2c, 2c+1); identical for all four row groups r
    w1sel_np = np.zeros((P, P), np.float32)
    for r in range(4):
        for s in range(NSTACK):
            for c in range(4):
                for jj in range(2):
                    j = 2 * c + jj
                    w1sel_np[32 * r + JBLK * s + j,
                             32 * c + HID * jj:32 * c + HID * (jj + 1)] = w1[s]
    # L2 weights: block-diagonal w2 per j (one copy; both column tiles load it)
    w2blk_np = np.zeros((P, JBLK * HEADS), np.float32)
    for j in range(JBLK):
        w2blk_np[HID * j:HID * (j + 1), HEADS * j:HEADS * (j + 1)] = w2
    idnil_np = np.eye(N, dtype=np_mm).reshape(NJB, JBLK, N)
    return {
        "w1sel": w1sel_np.astype(np_mm),
        "w2blk": w2blk_np.astype(np_mm),
        "b1rep": np.ascontiguousarray(np.tile(b1, JBLK).astype(np.float32)[:, None]),
        "idn32": np.eye(P, dtype=np.float32),
        "idnil": idnil_np,
    }


def kernel(adj, mask, w1, b1, w2, b2):
    from concourse.bass_utils import run_bass_kernel_spmd

    global LAST_RESULTS
    adj = np.ascontiguousarray(np.asarray(adj, dtype=np.float32))
    mask = np.asarray(mask)
    w1 = np.ascontiguousarray(np.asarray(w1, dtype=np.float32))
    b1 = np.ascontiguousarray(np.asarray(b1, dtype=np.float32))
    w2 = np.ascontiguousarray(np.asarray(w2, dtype=np.float32))
    b2 = np.asarray(b2, dtype=np.float32)
    assert adj.shape == (B, N, N), adj.shape

    m = mask.astype(np.float32)
    general_mask = not np.all(m == 1.0)
    if general_mask:
        pair = m[:, :, None] * m[:, None, :]
        adj = np.ascontiguousarray(adj * pair)

    trace = bool(int(os.environ.get("KERNEL_TRACE", "0")))
    if trace:
        _install_ntff_shim()
    mmname = os.environ.get("KERNEL_MM_DT", "float16")
    nc = _build(mmname)

    from concourse import mybir

    np_mm = mybir.dt.np(getattr(mybir.dt, mmname))
    shared = _host_tensors(w1, b1, w2, np_mm)
    # adj entries are exact in fp16 (0/1-masked values); pre-cast on the host
    # so the device load is a cheap non-casting HWDGE DMA
    adj_mm = np.ascontiguousarray(adj.astype(np_mm))
    in_maps = [{"adj": adj_mm[c], **shared} for c in range(B)]
    res = run_bass_kernel_spmd(nc, in_maps, list(range(B)), trace=trace)
    LAST_RESULTS = res

    outs = []
    for c in range(B):
        o2 = np.asarray(res.results[c]["out"])          # [64, 128, 1024] fp16
        o2 = o2.reshape(NJB // 2, 2, JBLK, HEADS, N)    # [pi, sub, j', o, i]
        o2 = np.transpose(o2, (4, 0, 1, 2, 3))          # [i, pi, sub, j', o]
        outs.append(o2.reshape(N, N, HEADS).astype(np.float32))
    outp = np.stack(outs, axis=0)

    if np.any(b2 != 0.0):
        outp = outp + b2
    if general_mask:
        outp = outp * pair[..., None]
    return np.ascontiguousarray(outp.astype(np.float32))


# revision 45
# speedup vs baseline: 1.0216x; 1.0216x over previous
"""Trainium2 Bass kernel for nn_Diffuser_78331613544465.

Math (per graph b of B=8, N=1024):
    A   = adj (mask is all-ones in the graded setup; general mask handled host-side)
    P   = A / max(rowsum(A), 1)
    out[i,j,:] = relu([I, P, P2, P4][i,j,:] @ w1 + b1) @ w2 + b2   (P2=P@P, P4=P2@P2)

Device strategy: data-parallel over B — one graph per NeuronCore (8 cores).
On-chip work happens in the TRANSPOSED domain (Q = P^T).  Because A is
symmetric, both P = D^-1 A (row scale) and Q = A D^-1 (col scale) come from
cheap elementwise scalings of A, and the power chain needs NO PE transposes:
    Q2 = P^T Q,  P2 = Q^T P,  Q4 = P2^T Q2      (matmul(lhsT=X, rhs=Y) = X^T Y)

Edge MLP layer 1 runs as 4 CONCURRENT row-tiled matmuls (tile_position=(32r,0),
K=32 = 4 stacks x 8 j's, M=128 = 8 j x 16 hid) per 512-i half — 8 matmuls per
32-j group instead of 32.  PSUM h tiles are [128,1024] (two banks, two j-blocks)
so the relu+bias evacuation runs at FD=1024, alternating Vector/Scalar (the only
two engines with PSUM access).  Layer 2 is K=128 block-diagonal with the two
j-blocks of a pair issued as concurrent column tiles (tile_position=(0,0)/(0,64)
against a duplicated weight [w2blk|w2blk]).

Channels are staged through a DRAM interleave il4[jb, s, jj, i] (jb = j-block,
s = 0:I (host-uploaded), 1:Q, 2:Q2, 3:Q4) laid out so that every group stage
load is one CONTIGUOUS 64KB DMA per j-block and every band spill is a simple
2-level affine pattern — no expensive gather descriptors.

The MLP groups are EMITTED INTERLEAVED into the second power-chain square with
a one-band lag, so the Vector/Scalar evacuation work (the MLP bottleneck)
overlaps the PE-bound power chain.

The [16j x 8o, i] PSUM result is evacuated as fp16 and the HOST un-transposes
— this keeps every output-DMA descriptor a full 2KB partition line.

kernel(**inputs) takes FULL inputs, shards over 8 cores, returns FULL output.
"""

import os
import numpy as np

B, N, P = 8, 1024, 128
HID, HEADS, NSTACK = 16, 8, 4
NT = N // P          # 8 row-tiles
JBLK = 8             # j rows per MLP block
NJB = N // JBLK      # 128 j-blocks
IC = 512             # i-chunk (matmul free dim)
NIC = N // IC        # 2
NGRP = N // 32       # 32 j-groups of 32 j's (4 j-blocks)

_CACHE = {}
LAST_RESULTS = None


def _emit(nc, tc, ctx, mm_dt):
    from concourse import mybir

    f32 = mybir.dt.float32
    add = mybir.AluOpType.add
    amax = mybir.AluOpType.max
    mult = mybir.AluOpType.mult
    relu_fn = mybir.ActivationFunctionType.Relu

    adj = nc.declare_dram_parameter("adj", [N, N], mm_dt, isOutput=False)
    w1sel_d = nc.declare_dram_parameter("w1sel", [P, P], mm_dt, isOutput=False)
    w2blk_d = nc.declare_dram_parameter("w2blk", [P, JBLK * HEADS], mm_dt, isOutput=False)
    b1rep_d = nc.declare_dram_parameter("b1rep", [P, 1], f32, isOutput=False)
    idn32_d = nc.declare_dram_parameter("idn32", [P, P], f32, isOutput=False)
    idnil_d = nc.declare_dram_parameter("idnil", [NJB, JBLK, N], mm_dt, isOutput=False)
    # device-natural output: [jb-pair, (16j x 8o) partition, i] in fp16;
    # host transposes to [i, j, o] and casts to f32
    out = nc.declare_dram_parameter("out", [NJB // 2, P, N], mm_dt, isOutput=True)

    small = ctx.enter_context(tc.tile_pool(name="small", bufs=1))
    big = ctx.enter_context(tc.tile_pool(name="big", bufs=1))
    spool = ctx.enter_context(tc.tile_pool(name="spool", bufs=8))
    rpool = ctx.enter_context(tc.tile_pool(name="rpool", bufs=20))
    ppool = ctx.enter_context(tc.tile_pool(name="ppool", bufs=8))
    dram = ctx.enter_context(tc.tile_pool(name="dram", bufs=1, space="DRAM"))
    # PSUM budget (8 banks): h 6x[128,512] single-bank slots (shared with the
    # power-chain accumulators; one slot per j-block so slots recycle
    # one-by-one instead of in 2-bank pairs), o 2x[128,512]=2
    h_ps = ctx.enter_context(tc.tile_pool(name="h_ps", bufs=6, space="PSUM"))
    o_ps = ctx.enter_context(tc.tile_pool(name="o_ps", bufs=2, space="PSUM"))

    # persistent matrices, one [128, 1024] tile per 128-row band
    Af = [big.tile([P, N], mm_dt, name=f"Af{t}", tag=f"Af{t}") for t in range(NT)]
    Pf = [big.tile([P, N], mm_dt, name=f"Pf{t}", tag=f"Pf{t}") for t in range(NT)]
    Qf = [big.tile([P, N], mm_dt, name=f"Qf{t}", tag=f"Qf{t}") for t in range(NT)]
    Q2f = [big.tile([P, N], mm_dt, name=f"Q2f{t}", tag=f"Q2f{t}") for t in range(NT)]
    P2f = [big.tile([P, N], mm_dt, name=f"P2f{t}", tag=f"P2f{t}") for t in range(NT)]
    Q4f = [big.tile([P, N], mm_dt, name=f"Q4f{t}", tag=f"Q4f{t}") for t in range(NT)]
    invrep = big.tile([P, N], f32, tag="invrep")
    # DRAM channel-interleave [jb, s, jj, i]: s=0 identity (host), 1..3=Q,Q2,Q4
    il4 = dram.tile([NJB, NSTACK, JBLK, N], mm_dt, tag="il4")

    # ---- constants / weights (host-prepared; one DMA each) -----------------
    idn32 = small.tile([P, P], f32, tag="idn32")
    nc.gpsimd.dma_start(idn32[:], idn32_d[:])
    ones1 = small.tile([1, P], f32, tag="ones1")
    nc.vector.memset(ones1[:], 1.0)
    w1sel = small.tile([P, P], mm_dt, tag="w1sel")
    nc.gpsimd.dma_start(w1sel[:], w1sel_d[:])
    w2blk = small.tile([P, JBLK * HEADS], mm_dt, tag="w2blk")
    nc.gpsimd.dma_start(w2blk[:], w2blk_d[:])
    b1rep = small.tile([P, 1], f32, tag="b1rep")
    nc.gpsimd.dma_start(b1rep[:], b1rep_d[:])

    # ---- phase 1: load adj (fp16 via DMA cast, 4 queue-spread chunks per
    # band), deg -> invdeg, P ------------------------------------------------
    invcol = small.tile([P, NT], f32, tag="invcol")
    for t in range(NT):
        # adj is pre-cast to fp16 by the host: cheap HWDGE loads, 2 queues/band
        for q in range(2):
            eng = nc.scalar if (2 * t + q) % 2 == 0 else nc.sync
            eng.dma_start(
                Af[t][:, 512 * q:512 * (q + 1)],
                adj[P * t:P * (t + 1), 512 * q:512 * (q + 1)],
            )
        deg = small.tile([P, 1], f32, tag=f"deg{t}")
        nc.vector.tensor_reduce(
            deg[:], Af[t][:], axis=mybir.AxisListType.X, op=add,
        )
        degc = small.tile([P, 1], f32, tag=f"degc{t}")
        nc.vector.tensor_scalar_max(degc[:], deg[:], 1.0)
        nc.vector.reciprocal(invcol[:, t:t + 1], degc[:])
        # P = A * invdeg[row]  (per-partition scale on the scalar engine)
        nc.scalar.mul(Pf[t][:], Af[t][:], invcol[:, t:t + 1])

    # identity channel of the interleave: 16 chunked DRAM->DRAM copies,
    # emitted AFTER the adj loads so they don't delay band arrival (only
    # needed once the MLP stage loads begin)
    for ch in range(16):
        nc.sync.dma_start(
            il4[8 * ch:8 * (ch + 1), 0:1, :, :],
            idnil_d[8 * ch:8 * (ch + 1), :, :],
        )

    # invrep[p, c] = invdeg(row c) for all p  (transpose + broadcast via PE)
    invrow = small.tile([1, N], f32, tag="invrow")
    for t in range(NT):
        ptp = o_ps.tile([P, IC], f32, tag="O")
        nc.tensor.transpose(ptp[0:1, 0:P], invcol[:, t:t + 1], idn32[:])
        nc.scalar.copy(invrow[0:1, P * t:P * (t + 1)], ptp[0:1, 0:P])
    for half in range(2):
        pb = o_ps.tile([P, IC], f32, tag="O")
        for k in range(4):
            c = 4 * half + k
            nc.tensor.matmul(
                pb[:, P * k:P * (k + 1)], ones1[:], invrow[0:1, P * c:P * (c + 1)],
                start=True, stop=True,
            )
        nc.scalar.copy(invrep[:, IC * half:IC * (half + 1)], pb[:])

    def spill(si, t, src, split=False):
        # optional split: two half-band DMAs on separate queues halve the
        # DRAM-visibility latency (used for Q4, which gates the MLP stage)
        if split:
            nc.sync.dma_start(
                il4[16 * t:16 * t + 8, si:si + 1, :, :], src[0:64, :])
            nc.sync.dma_start(
                il4[16 * t + 8:16 * t + 16, si:si + 1, :, :], src[64:128, :])
        else:
            nc.sync.dma_start(il4[16 * t:16 * (t + 1), si:si + 1, :, :], src[:])

    # PE warmup: keep the HAM activity monitor hot so the first square runs
    # at 2.4 GHz from its first matmul (overlaps the tail of the adj loads)
    warm = o_ps.tile([P, IC], f32, tag="O")
    for i in range(20):
        nc.tensor.matmul(
            warm[:], Af[0][:, 0:P], Af[0][:, 0:IC],
            start=(i == 0), stop=(i == 19),
        )

    # ---- power chain (no transposes; M3 := A D^-1 A is symmetric, so ONE
    # square yields both Q2 = M3 D^-1 (col scale) and P2 = D^-1 M3 (row
    # scale); then Q4 = Q2^2 = P2^T Q2) -------------------------------------
    for al in range(NT):
        for be in range(NIC):
            mm = h_ps.tile([P, IC], f32, tag="H")
            for g in range(NT):
                nc.tensor.matmul(
                    mm[:],
                    Af[g][:, P * al:P * (al + 1)],
                    Pf[g][:, IC * be:IC * (be + 1)],
                    start=(g == 0), stop=(g == NT - 1),
                )
            nc.vector.tensor_tensor(
                Q2f[al][:, IC * be:IC * (be + 1)], mm[:],
                invrep[:, IC * be:IC * (be + 1)], op=mult,
            )
            nc.scalar.mul(
                P2f[al][:, IC * be:IC * (be + 1)], mm[:], invcol[:, al:al + 1],
            )
        spill(2, al, Q2f[al])

    # ---- edge MLP, software-pipelined over groups (32 j's each) ------------
    stage_tiles = {}
    rt_tiles = {}

    def emit_stage(G):
        stage = spool.tile([P, N], mm_dt, tag="S")
        # ONE contiguous 256KB DMA per group: partitions (r, s, jj)
        eng = nc.sync if G % 2 == 0 else nc.scalar
        eng.dma_start(stage[:], il4[4 * G:4 * (G + 1), :, :, :])
        stage_tiles[G] = stage

    def emit_l1(G):
        stage = stage_tiles.pop(G)
        rts = {}
        for ic in range(NIC):
            # 16 concurrent 32x32 PE tiles (the only tile size that packs):
            # tile (r,c) computes j-pair (2c,2c+1) x 16 hid of block 4G+r,
            # one single-bank PSUM slot per j-block r
            hs = [h_ps.tile([P, IC], f32, name=f"h{r}", tag="H")
                  for r in range(4)]
            for r in range(4):
                for c2 in range(2):
                    nc.tensor.matmul(
                        hs[r][64 * c2:64 * (c2 + 1), :],
                        w1sel[32 * r:32 * (r + 1), 64 * c2:64 * (c2 + 1)],
                        stage[32 * r:32 * (r + 1), IC * ic:IC * (ic + 1)],
                        start=True, stop=True, tile_position=(32 * r, 64 * c2),
                    )
            # relu+bias evacuation per block, alternating V/S so slots free
            # one-by-one on both engines
            for r in range(4):
                rt = rpool.tile([P, IC], mm_dt, name=f"rt{r}", tag="R")
                if r % 2 == 0:
                    nc.vector.tensor_scalar(rt[:], hs[r][:], b1rep[:], 0.0,
                                            add, amax)
                else:
                    nc.scalar.activation(rt[:], hs[r][:], relu_fn,
                                         bias=b1rep[:], scale=1.0)
                rts[(r, ic)] = rt
        rt_tiles[G] = rts

    def emit_l2(G):
        rts = rt_tiles.pop(G)
        for rp in range(2):
            psout = ppool.tile([P, N], mm_dt, tag="PS")
            pos = [o_ps.tile([P, IC], f32, name=f"po{i}", tag="O")
                   for i in range(NIC)]
            # emission order A(ic0), A(ic1), B(ic0), B(ic1): consecutive MMs
            # share lhsT AP and tile_position, giving LDW reuse a chance
            nc.tensor.matmul(
                pos[0][0:64, :], w2blk[:], rts[(2 * rp, 0)][:],
                start=True, stop=True, tile_position=(0, 0),
            )
            nc.tensor.matmul(
                pos[1][0:64, :], w2blk[:], rts[(2 * rp, 1)][:],
                start=True, stop=True, tile_position=(0, 0),
            )
            nc.tensor.matmul(
                pos[0][64:128, :], w2blk[:], rts[(2 * rp + 1, 0)][:],
                start=True, stop=True, tile_position=(0, 64),
            )
            nc.tensor.matmul(
                pos[1][64:128, :], w2blk[:], rts[(2 * rp + 1, 1)][:],
                start=True, stop=True, tile_position=(0, 64),
            )
            for ic in range(NIC):
                # split the 4 po evacs 2/2 across V and S
                if (rp + ic) % 2 == 0:
                    nc.vector.tensor_scalar_add(
                        psout[:, IC * ic:IC * (ic + 1)], pos[ic][:], 0.0,
                    )
                else:
                    nc.scalar.copy(psout[:, IC * ic:IC * (ic + 1)], pos[ic][:])
            # out DMA on gpsimd: idle during the MLP and its (possibly
            # waiting) trigger blocks nothing else
            nc.gpsimd.dma_start(out[2 * G + rp], psout[:])

    prog = [0, 0, 0]  # stage / l1 / l2 emission pointers

    def pump(stage_t, l1_t, l2_t):
        # round-robin the three pipeline stages up to their targets
        while prog[0] < stage_t or prog[1] < l1_t or prog[2] < l2_t:
            if prog[0] < stage_t:
                emit_stage(prog[0])
                prog[0] += 1
            if prog[1] < l1_t:
                emit_l1(prog[1])
                prog[1] += 1
            if prog[2] < l2_t:
                emit_l2(prog[2])
                prog[2] += 1

    # ---- second square, MLP pipeline interleaved as bands become ready -----
    for al in range(NT):
        for be in range(NIC):
            mm = h_ps.tile([P, IC], f32, tag="H")
            for g in range(NT):
                nc.tensor.matmul(
                    mm[:],
                    P2f[g][:, P * al:P * (al + 1)],
                    Q2f[g][:, IC * be:IC * (be + 1)],
                    start=(g == 0), stop=(g == NT - 1),
                )
            if be == 0:
                nc.vector.tensor_scalar_add(Q4f[al][:, 0:IC], mm[:], 0.0)
            else:
                nc.scalar.copy(Q4f[al][:, IC:N], mm[:])
        spill(3, al, Q4f[al], split=True)
        # Q channel for band al: V has slack during the second square, and
        # doing it here keeps the first square's scale consumers unblocked
        nc.vector.tensor_tensor(Qf[al][:], Af[al][:], invrep[:], op=mult)
        spill(1, al, Qf[al])
        # groups of band al are now ready: prefetch their stage DMAs (pure
        # background traffic), but keep ALL matmul/evac work out of the
        # power chain - interleaving couples the phases through PSUM slots
        # and breaks correctness (observed 7e-2 rel err when L1 waves share
        # the pool with in-flight accumulations)
        pump(min(4 * (al + 1), NGRP), 0, 0)
    pump(NGRP, NGRP, NGRP)


def _build(mm_dtype_name="float16"):
    key = mm_dtype_name
    if key in _CACHE:
        return _CACHE[key]
    from contextlib import ExitStack
    import concourse.tile as tile
    from concourse import bacc, mybir

    nc = bacc.Bacc()
    with tile.TileContext(nc) as tc:
        with ExitStack() as ctx:
            _emit(nc, tc, ctx, getattr(mybir.dt, mm_dtype_name))
    nc.compile()
    _CACHE[key] = nc
    return nc


def _install_ntff_shim():
    """The agent image's antenv lacks axon_hooks; provide it and register the
    ctypes NTFF hook so run_bass_kernel_spmd(trace=True) can profile."""
    import sys
    import types

    if "antenv.axon_hooks" in sys.modules:
        return
    mod = types.ModuleType("antenv.axon_hooks")
    mod._hook = None
    mod.set_axon_ntff_profile_hook = lambda h: setattr(mod, "_hook", h)
    mod.get_axon_ntff_profile_hook = lambda: mod._hook
    sys.modules["antenv.axon_hooks"] = mod
    try:
        from trn_agent_boot.trn_boot import _ntff_profile_via_ctypes

        mod._hook = _ntff_profile_via_ctypes("/opt/axon/libaxon_pjrt.so")
    except Exception as e:  # degrade to no-trace
        print(f"ntff shim install failed: {e}")


def _host_tensors(w1, b1, w2, np_mm):
    # selector weights: tile (r,c) maps staged rows (s, jj) -> (jj', hid) of
    # j-pair (2c, 2c+1); identical for all four row groups r
    w1sel_np = np.zeros((P, P), np.float32)
    for r in range(4):
        for s in range(NSTACK):
            for c in range(4):
                for jj in range(2):
                    j = 2 * c + jj
                    w1sel_np[32 * r + JBLK * s + j,
                             32 * c + HID * jj:32 * c + HID * (jj + 1)] = w1[s]
    # L2 weights: block-diagonal w2 per j (one copy; both column tiles load it)
    w2blk_np = np.zeros((P, JBLK * HEADS), np.float32)
    for j in range(JBLK):
        w2blk_np[HID * j:HID * (j + 1), HEADS * j:HEADS * (j + 1)] = w2
    idnil_np = np.eye(N, dtype=np_mm).reshape(NJB, JBLK, N)
    return {
        "w1sel": w1sel_np.astype(np_mm),
        "w2blk": w2blk_np.astype(np_mm),
        "b1rep": np.ascontiguousarray(np.tile(b1, JBLK).astype(np.float32)[:, None]),
        "idn32": np.eye(P, dtype=np.float32),
        "idnil": idnil_np,
    }


def kernel(adj, mask, w1, b1, w2, b2):
    from concourse.bass_utils import run_bass_kernel_spmd

    global LAST_RESULTS
    adj = np.ascontiguousarray(np.asarray(adj, dtype=np.float32))
    mask = np.asarray(mask)
    w1 = np.ascontiguousarray(np.asarray(w1, dtype=np.float32))
    b1 = np.ascontiguousarray(np.asarray(b1, dtype=np.float32))
    w2 = np.ascontiguousarray(np.asarray(w2, dtype=np.float32))
    b2 = np.asarray(b2, dtype=np.float32)
    assert adj.shape == (B, N, N), adj.shape

    m = mask.astype(np.float32)
    general_mask = not np.all(m == 1.0)
    if general_mask:
        pair = m[:, :, None] * m[:, None, :]
        adj = np.ascontiguousarray(adj * pair)

    trace = bool(int(os.environ.get("KERNEL_TRACE", "0")))
    if trace:
        _install_ntff_shim()
    mmname = os.environ.get("KERNEL_MM_DT", "float16")
    nc = _build(mmname)

    from concourse import mybir

    np_mm = mybir.dt.np(getattr(mybir.dt, mmname))
    shared = _host_tensors(w1, b1, w2, np_mm)
    # adj entries are exact in fp16 (0/1-masked values); pre-cast on the host
    # so the device load is a cheap non-casting HWDGE DMA
    adj_mm = np.ascontiguousarray(adj.astype(np_mm))
    in_maps = [{"adj": adj_mm[c], **shared} for c in range(B)]
    res = run_bass_kernel_spmd(nc, in_maps, list(range(B)), trace=trace)
    LAST_RESULTS = res

    outs = []
    for c in range(B):
        o2 = np.asarray(res.results[c]["out"])          # [64, 128, 1024] fp16
        o2 = o2.reshape(NJB // 2, 2, JBLK, HEADS, N)    # [pi, sub, j', o, i]
        o2 = np.transpose(o2, (4, 0, 1, 2, 3))          # [i, pi, sub, j', o]
        outs.append(o2.reshape(N, N, HEADS).astype(np.float32))
    outp = np.stack(outs, axis=0)

    if np.any(b2 != 0.0):
        outp = outp + b2
    if general_mask:
        outp = outp * pair[..., None]
    return np.ascontiguousarray(outp.astype(np.float32))


# revision 46
# speedup vs baseline: 1.0315x; 1.0097x over previous
"""Trainium2 Bass kernel for nn_Diffuser_78331613544465.

Math (per graph b of B=8, N=1024):
    A   = adj (mask is all-ones in the graded setup; general mask handled host-side)
    P   = A / max(rowsum(A), 1)
    out[i,j,:] = relu([I, P, P2, P4][i,j,:] @ w1 + b1) @ w2 + b2   (P2=P@P, P4=P2@P2)

Device strategy: data-parallel over B — one graph per NeuronCore (8 cores).
On-chip work happens in the TRANSPOSED domain (Q = P^T).  Because A is
symmetric, both P = D^-1 A (row scale) and Q = A D^-1 (col scale) come from
cheap elementwise scalings of A, and the power chain needs NO PE transposes:
    Q2 = P^T Q,  P2 = Q^T P,  Q4 = P2^T Q2      (matmul(lhsT=X, rhs=Y) = X^T Y)

Edge MLP layer 1 runs as 8 CONCURRENT 32x64 PE tiles per 512-i wave
(tile_position=(32r, 64c2), K=32 = 4 stacks x 8 j's, M=64 = 4 j x 16 hid):
tile (r, c2) writes single-bank PSUM slot hs[r] partitions 64c2..64c2+64.
Six single-bank h slots (+2 for layer-2 po) let slots recycle one at a time
— the [128,1024] 2-bank variant structurally fragments every pack 8+8.
The relu+bias evacuation runs at FD=512 per j-block, alternating
Vector/Scalar (the only two engines with PSUM access; GpSimd and DMA cannot
touch PSUM).  Layer 2 is K=128 block-diagonal, column-tile pair
(tile_position=(0,0)/(0,64)) with the same w2blk lhsT AP so back-to-back
LDWEIGHTS dedup.

Channels are staged through a DRAM interleave il4[jb, s, jj, i] (jb = j-block,
s=0 identity filled once from the host param via 16 chunked DMAs AFTER the adj
loads, 1..3 = Q,Q2,Q4 spilled per band) so every group stage load is ONE
contiguous 256KB DMA and every spill a simple 2-level affine pattern.  The Q
channel is computed inside the second square (Vector has slack there; a
GpSimd/Vector split runs 2.5x slower from SBUF port sharing).  All hot DMA
triggers ride the two HWDGE queues (sync/scalar) or idle gpsimd (out);
trigger count is a first-class cost (~650ns each).

Only stage DMAs are prefetched into the power-chain window: interleaving MLP
compute into the second square couples the phases through PSUM slots (slow)
and was observed to corrupt results.

The [16j x 8o, i] PSUM result is evacuated as fp16 and the HOST un-transposes
— this keeps every output-DMA descriptor a full 2KB partition line.

kernel(**inputs) takes FULL inputs, shards over 8 cores, returns FULL output.
"""

import os
import numpy as np

B, N, P = 8, 1024, 128
HID, HEADS, NSTACK = 16, 8, 4
NT = N // P          # 8 row-tiles
JBLK = 8             # j rows per MLP block
NJB = N // JBLK      # 128 j-blocks
IC = 512             # i-chunk (matmul free dim)
NIC = N // IC        # 2
NGRP = N // 32       # 32 j-groups of 32 j's (4 j-blocks)

_CACHE = {}
LAST_RESULTS = None


def _emit(nc, tc, ctx, mm_dt):
    from concourse import mybir

    f32 = mybir.dt.float32
    add = mybir.AluOpType.add
    amax = mybir.AluOpType.max
    mult = mybir.AluOpType.mult
    relu_fn = mybir.ActivationFunctionType.Relu

    adj = nc.declare_dram_parameter("adj", [N, N], mm_dt, isOutput=False)
    w1sel_d = nc.declare_dram_parameter("w1sel", [P, P], mm_dt, isOutput=False)
    w2blk_d = nc.declare_dram_parameter("w2blk", [P, JBLK * HEADS], mm_dt, isOutput=False)
    b1rep_d = nc.declare_dram_parameter("b1rep", [P, 1], f32, isOutput=False)
    idn32_d = nc.declare_dram_parameter("idn32", [P, P], f32, isOutput=False)
    idnil_d = nc.declare_dram_parameter("idnil", [NJB, JBLK, N], mm_dt, isOutput=False)
    # device-natural output: [jb-pair, (16j x 8o) partition, i] in fp16;
    # host transposes to [i, j, o] and casts to f32
    out = nc.declare_dram_parameter("out", [NJB // 2, P, N], mm_dt, isOutput=True)

    small = ctx.enter_context(tc.tile_pool(name="small", bufs=1))
    big = ctx.enter_context(tc.tile_pool(name="big", bufs=1))
    spool = ctx.enter_context(tc.tile_pool(name="spool", bufs=8))
    rpool = ctx.enter_context(tc.tile_pool(name="rpool", bufs=20))
    ppool = ctx.enter_context(tc.tile_pool(name="ppool", bufs=8))
    dram = ctx.enter_context(tc.tile_pool(name="dram", bufs=1, space="DRAM"))
    # PSUM budget (8 banks): h 6x[128,512] single-bank slots (shared with the
    # power-chain accumulators; one slot per j-block so slots recycle
    # one-by-one instead of in 2-bank pairs), o 2x[128,512]=2
    h_ps = ctx.enter_context(tc.tile_pool(name="h_ps", bufs=6, space="PSUM"))
    o_ps = ctx.enter_context(tc.tile_pool(name="o_ps", bufs=2, space="PSUM"))

    # persistent matrices, one [128, 1024] tile per 128-row band
    Af = [big.tile([P, N], mm_dt, name=f"Af{t}", tag=f"Af{t}") for t in range(NT)]
    Pf = [big.tile([P, N], mm_dt, name=f"Pf{t}", tag=f"Pf{t}") for t in range(NT)]
    Qf = [big.tile([P, N], mm_dt, name=f"Qf{t}", tag=f"Qf{t}") for t in range(NT)]
    Q2f = [big.tile([P, N], mm_dt, name=f"Q2f{t}", tag=f"Q2f{t}") for t in range(NT)]
    P2f = [big.tile([P, N], mm_dt, name=f"P2f{t}", tag=f"P2f{t}") for t in range(NT)]
    Q4f = [big.tile([P, N], mm_dt, name=f"Q4f{t}", tag=f"Q4f{t}") for t in range(NT)]
    invrep = big.tile([P, N], f32, tag="invrep")
    # DRAM channel-interleave [jb, s, jj, i]: s=0 identity (host), 1..3=Q,Q2,Q4
    il4 = dram.tile([NJB, NSTACK, JBLK, N], mm_dt, tag="il4")

    # ---- constants / weights (host-prepared; one DMA each) -----------------
    idn32 = small.tile([P, P], f32, tag="idn32")
    nc.gpsimd.dma_start(idn32[:], idn32_d[:])
    ones1 = small.tile([1, P], f32, tag="ones1")
    nc.vector.memset(ones1[:], 1.0)
    w1sel = small.tile([P, P], mm_dt, tag="w1sel")
    nc.gpsimd.dma_start(w1sel[:], w1sel_d[:])
    w2blk = small.tile([P, JBLK * HEADS], mm_dt, tag="w2blk")
    nc.gpsimd.dma_start(w2blk[:], w2blk_d[:])
    b1rep = small.tile([P, 1], f32, tag="b1rep")
    nc.gpsimd.dma_start(b1rep[:], b1rep_d[:])

    # ---- phase 1: load adj (fp16 via DMA cast, 4 queue-spread chunks per
    # band), deg -> invdeg, P ------------------------------------------------
    invcol = small.tile([P, NT], f32, tag="invcol")
    for t in range(NT):
        # adj is pre-cast to fp16 by the host: cheap HWDGE loads, 2 queues/band
        for q in range(2):
            eng = nc.scalar if (2 * t + q) % 2 == 0 else nc.sync
            eng.dma_start(
                Af[t][:, 512 * q:512 * (q + 1)],
                adj[P * t:P * (t + 1), 512 * q:512 * (q + 1)],
            )
        deg = small.tile([P, 1], f32, tag=f"deg{t}")
        nc.vector.tensor_reduce(
            deg[:], Af[t][:], axis=mybir.AxisListType.X, op=add,
        )
        degc = small.tile([P, 1], f32, tag=f"degc{t}")
        nc.vector.tensor_scalar_max(degc[:], deg[:], 1.0)
        nc.vector.reciprocal(invcol[:, t:t + 1], degc[:])
        # P = A * invdeg[row]  (per-partition scale on the scalar engine)
        nc.scalar.mul(Pf[t][:], Af[t][:], invcol[:, t:t + 1])

    # identity channel of the interleave: 16 chunked DRAM->DRAM copies,
    # emitted AFTER the adj loads so they don't delay band arrival (only
    # needed once the MLP stage loads begin)
    for ch in range(16):
        nc.sync.dma_start(
            il4[8 * ch:8 * (ch + 1), 0:1, :, :],
            idnil_d[8 * ch:8 * (ch + 1), :, :],
        )

    # invrep[p, c] = invdeg(row c) for all p  (transpose + broadcast via PE)
    invrow = small.tile([1, N], f32, tag="invrow")
    for t in range(NT):
        ptp = o_ps.tile([P, IC], f32, tag="O")
        nc.tensor.transpose(ptp[0:1, 0:P], invcol[:, t:t + 1], idn32[:])
        nc.scalar.copy(invrow[0:1, P * t:P * (t + 1)], ptp[0:1, 0:P])
    for half in range(2):
        pb = o_ps.tile([P, IC], f32, tag="O")
        for k in range(4):
            c = 4 * half + k
            nc.tensor.matmul(
                pb[:, P * k:P * (k + 1)], ones1[:], invrow[0:1, P * c:P * (c + 1)],
                start=True, stop=True,
            )
        nc.scalar.copy(invrep[:, IC * half:IC * (half + 1)], pb[:])

    def spill(si, t, src, split=False):
        # optional split: two half-band DMAs on separate queues halve the
        # DRAM-visibility latency (used for Q4, which gates the MLP stage)
        if split:
            nc.sync.dma_start(
                il4[16 * t:16 * t + 8, si:si + 1, :, :], src[0:64, :])
            nc.sync.dma_start(
                il4[16 * t + 8:16 * t + 16, si:si + 1, :, :], src[64:128, :])
        else:
            nc.sync.dma_start(il4[16 * t:16 * (t + 1), si:si + 1, :, :], src[:])

    # PE warmup: keep the HAM activity monitor hot so the first square runs
    # at 2.4 GHz from its first matmul (overlaps the tail of the adj loads)
    warm = o_ps.tile([P, IC], f32, tag="O")
    for i in range(20):
        nc.tensor.matmul(
            warm[:], Af[0][:, 0:P], Af[0][:, 0:IC],
            start=(i == 0), stop=(i == 19),
        )

    # ---- power chain (no transposes; M3 := A D^-1 A is symmetric, so ONE
    # square yields both Q2 = M3 D^-1 (col scale) and P2 = D^-1 M3 (row
    # scale); then Q4 = Q2^2 = P2^T Q2) -------------------------------------
    for al in range(NT):
        for be in range(NIC):
            mm = h_ps.tile([P, IC], f32, tag="H")
            for g in range(NT):
                nc.tensor.matmul(
                    mm[:],
                    Af[g][:, P * al:P * (al + 1)],
                    Pf[g][:, IC * be:IC * (be + 1)],
                    start=(g == 0), stop=(g == NT - 1),
                )
            nc.vector.tensor_tensor(
                Q2f[al][:, IC * be:IC * (be + 1)], mm[:],
                invrep[:, IC * be:IC * (be + 1)], op=mult,
            )
            nc.scalar.mul(
                P2f[al][:, IC * be:IC * (be + 1)], mm[:], invcol[:, al:al + 1],
            )
        spill(2, al, Q2f[al])

    # ---- edge MLP, software-pipelined over groups (32 j's each) ------------
    stage_tiles = {}
    rt_tiles = {}

    def emit_stage(G):
        stage = spool.tile([P, N], mm_dt, tag="S")
        # ONE contiguous 256KB DMA per group: partitions (r, s, jj)
        eng = nc.sync if G % 2 == 0 else nc.scalar
        eng.dma_start(stage[:], il4[4 * G:4 * (G + 1), :, :, :])
        stage_tiles[G] = stage

    def emit_l1(G):
        stage = stage_tiles.pop(G)
        rts = {}
        for ic in range(NIC):
            # 16 concurrent 32x32 PE tiles (the only tile size that packs):
            # tile (r,c) computes j-pair (2c,2c+1) x 16 hid of block 4G+r,
            # one single-bank PSUM slot per j-block r
            hs = [h_ps.tile([P, IC], f32, name=f"h{r}", tag="H")
                  for r in range(4)]
            for r in range(4):
                for c2 in range(2):
                    nc.tensor.matmul(
                        hs[r][64 * c2:64 * (c2 + 1), :],
                        w1sel[32 * r:32 * (r + 1), 64 * c2:64 * (c2 + 1)],
                        stage[32 * r:32 * (r + 1), IC * ic:IC * (ic + 1)],
                        start=True, stop=True, tile_position=(32 * r, 64 * c2),
                    )
            # relu+bias evacuation per block, alternating V/S so slots free
            # one-by-one on both engines
            for r in range(4):
                rt = rpool.tile([P, IC], mm_dt, name=f"rt{r}", tag="R")
                if r % 2 == 0:
                    nc.vector.tensor_scalar(rt[:], hs[r][:], b1rep[:], 0.0,
                                            add, amax)
                else:
                    nc.scalar.activation(rt[:], hs[r][:], relu_fn,
                                         bias=b1rep[:], scale=1.0)
                rts[(r, ic)] = rt
        rt_tiles[G] = rts

    def emit_l2(G):
        rts = rt_tiles.pop(G)
        for rp in range(2):
            psout = ppool.tile([P, N], mm_dt, tag="PS")
            pos = [o_ps.tile([P, IC], f32, name=f"po{i}", tag="O")
                   for i in range(NIC)]
            # emission order A(ic0), A(ic1), B(ic0), B(ic1): consecutive MMs
            # share lhsT AP and tile_position, giving LDW reuse a chance
            nc.tensor.matmul(
                pos[0][0:64, :], w2blk[:], rts[(2 * rp, 0)][:],
                start=True, stop=True, tile_position=(0, 0),
            )
            nc.tensor.matmul(
                pos[1][0:64, :], w2blk[:], rts[(2 * rp, 1)][:],
                start=True, stop=True, tile_position=(0, 0),
            )
            nc.tensor.matmul(
                pos[0][64:128, :], w2blk[:], rts[(2 * rp + 1, 0)][:],
                start=True, stop=True, tile_position=(0, 64),
            )
            nc.tensor.matmul(
                pos[1][64:128, :], w2blk[:], rts[(2 * rp + 1, 1)][:],
                start=True, stop=True, tile_position=(0, 64),
            )
            for ic in range(NIC):
                # split the 4 po evacs 2/2 across V and S
                if (rp + ic) % 2 == 0:
                    nc.vector.tensor_scalar_add(
                        psout[:, IC * ic:IC * (ic + 1)], pos[ic][:], 0.0,
                    )
                else:
                    nc.scalar.copy(psout[:, IC * ic:IC * (ic + 1)], pos[ic][:])
            # out DMA on gpsimd: idle during the MLP and its (possibly
            # waiting) trigger blocks nothing else
            nc.gpsimd.dma_start(out[2 * G + rp], psout[:])

    prog = [0, 0, 0]  # stage / l1 / l2 emission pointers

    def pump(stage_t, l1_t, l2_t):
        # round-robin the three pipeline stages up to their targets
        while prog[0] < stage_t or prog[1] < l1_t or prog[2] < l2_t:
            if prog[0] < stage_t:
                emit_stage(prog[0])
                prog[0] += 1
            if prog[1] < l1_t:
                emit_l1(prog[1])
                prog[1] += 1
            if prog[2] < l2_t:
                emit_l2(prog[2])
                prog[2] += 1

    # ---- second square, MLP pipeline interleaved as bands become ready -----
    for al in range(NT):
        for be in range(NIC):
            mm = h_ps.tile([P, IC], f32, tag="H")
            for g in range(NT):
                nc.tensor.matmul(
                    mm[:],
                    P2f[g][:, P * al:P * (al + 1)],
                    Q2f[g][:, IC * be:IC * (be + 1)],
                    start=(g == 0), stop=(g == NT - 1),
                )
            if be == 0:
                nc.vector.tensor_scalar_add(Q4f[al][:, 0:IC], mm[:], 0.0)
            else:
                nc.scalar.copy(Q4f[al][:, IC:N], mm[:])
        spill(3, al, Q4f[al], split=True)
        # Q channel for band al: V has slack during the second square, and
        # doing it here keeps the first square's scale consumers unblocked
        nc.vector.tensor_tensor(Qf[al][:], Af[al][:], invrep[:], op=mult)
        spill(1, al, Qf[al])
        # groups of band al are now ready: prefetch their stage DMAs (pure
        # background traffic), but keep ALL matmul/evac work out of the
        # power chain - interleaving couples the phases through PSUM slots
        # and breaks correctness (observed 7e-2 rel err when L1 waves share
        # the pool with in-flight accumulations)
        pump(min(4 * (al + 1), NGRP), 0, 0)
    pump(NGRP, NGRP, NGRP)


def _build(mm_dtype_name="float16"):
    key = mm_dtype_name
    if key in _CACHE:
        return _CACHE[key]
    from contextlib import ExitStack
    import concourse.tile as tile
    from concourse import bacc, mybir

    nc = bacc.Bacc()
    with tile.TileContext(nc) as tc:
        with ExitStack() as ctx:
            _emit(nc, tc, ctx, getattr(mybir.dt, mm_dtype_name))
    nc.compile()
    _CACHE[key] = nc
    return nc


def _install_ntff_shim():
    """The agent image's antenv lacks axon_hooks; provide it and register the
    ctypes NTFF hook so run_bass_kernel_spmd(trace=True) can profile."""
    import sys
    import types

    if "antenv.axon_hooks" in sys.modules:
        return
    mod = types.ModuleType("antenv.axon_hooks")
    mod._hook = None
    mod.set_axon_ntff_profile_hook = lambda h: setattr(mod, "_hook", h)
    mod.get_axon_ntff_profile_hook = lambda: mod._hook
    sys.modules["antenv.axon_hooks"] = mod
    try:
        from trn_agent_boot.trn_boot import _ntff_profile_via_ctypes

        mod._hook = _ntff_profile_via_ctypes("/opt/axon/libaxon_pjrt.so")
    except Exception as e:  # degrade to no-trace
        print(f"ntff shim install failed: {e}")


def _host_tensors(w1, b1, w2, np_mm):
    # selector weights: tile (r,c) maps staged rows (s, jj) -> (jj', hid) of
    # j-pair (2c, 2c+1); identical for all four row groups r
    w1sel_np = np.zeros((P, P), np.float32)
    for r in range(4):
        for s in range(NSTACK):
            for c in range(4):
                for jj in range(2):
                    j = 2 * c + jj
                    w1sel_np[32 * r + JBLK * s + j,
                             32 * c + HID * jj:32 * c + HID * (jj + 1)] = w1[s]
    # L2 weights: block-diagonal w2 per j (one copy; both column tiles load it)
    w2blk_np = np.zeros((P, JBLK * HEADS), np.float32)
    for j in range(JBLK):
        w2blk_np[HID * j:HID * (j + 1), HEADS * j:HEADS * (j + 1)] = w2
    idnil_np = np.eye(N, dtype=np_mm).reshape(NJB, JBLK, N)
    return {
        "w1sel": w1sel_np.astype(np_mm),
        "w2blk": w2blk_np.astype(np_mm),
        "b1rep": np.ascontiguousarray(np.tile(b1, JBLK).astype(np.float32)[:, None]),
        "idn32": np.eye(P, dtype=np.float32),
        "idnil": idnil_np,
    }


def kernel(adj, mask, w1, b1, w2, b2):
    from concourse.bass_utils import run_bass_kernel_spmd

    global LAST_RESULTS
    adj = np.ascontiguousarray(np.asarray(adj, dtype=np.float32))
    mask = np.asarray(mask)
    w1 = np.ascontiguousarray(np.asarray(w1, dtype=np.float32))
    b1 = np.ascontiguousarray(np.asarray(b1, dtype=np.float32))
    w2 = np.ascontiguousarray(np.asarray(w2, dtype=np.float32))
    b2 = np.asarray(b2, dtype=np.float32)
    assert adj.shape == (B, N, N), adj.shape

    m = mask.astype(np.float32)
    general_mask = not np.all(m == 1.0)
    if general_mask:
        pair = m[:, :, None] * m[:, None, :]
        adj = np.ascontiguousarray(adj * pair)

    trace = bool(int(os.environ.get("KERNEL_TRACE", "0")))
    if trace:
        _install_ntff_shim()
    mmname = os.environ.get("KERNEL_MM_DT", "float16")
    nc = _build(mmname)

    from concourse import mybir

    np_mm = mybir.dt.np(getattr(mybir.dt, mmname))
    shared = _host_tensors(w1, b1, w2, np_mm)
    # adj entries are exact in fp16 (0/1-masked values); pre-cast on the host
    # so the device load is a cheap non-casting HWDGE DMA
    adj_mm = np.ascontiguousarray(adj.astype(np_mm))
    in_maps = [{"adj": adj_mm[c], **shared} for c in range(B)]
    res = run_bass_kernel_spmd(nc, in_maps, list(range(B)), trace=trace)
    LAST_RESULTS = res

    outs = []
    for c in range(B):
        o2 = np.asarray(res.results[c]["out"])          # [64, 128, 1024] fp16
        o2 = o2.reshape(NJB // 2, 2, JBLK, HEADS, N)    # [pi, sub, j', o, i]
        o2 = np.transpose(o2, (4, 0, 1, 2, 3))          # [i, pi, sub, j', o]
        outs.append(o2.reshape(N, N, HEADS).astype(np.float32))
    outp = np.stack(outs, axis=0)

    if np.any(b2 != 0.0):
        outp = outp + b2
    if general_mask:
        outp = outp * pair[..., None]
    return np.ascontiguousarray(outp.astype(np.float32))
